# revision 11
# baseline (speedup 1.0000x reference)
"""Trainium2 Bass kernel for nn_Detect (YOLO-style heads + ROI-pooled obj features).

Sharding: data-parallel over batch — core b owns image b. Boxes are selected
per-core via an on-device indicator (labels[:,0] == core_id); the per-core obj
partial outputs are disjoint row-wise and summed on the host (unshard).

Math: the reference's upsample(bilinear/bicubic to 100x100) -> SAT -> box-sum
pipeline is linear and separable, so per box n and level l:
    obj[n, c] = (1/area_n) * r_n^T  X_l[b_n, c]  c_n
with r_n = A_l^T m_y_n, c_n = A_l^T m_x_n, where A_l is the (constant) 1-D
interpolation matrix [100, H_l] and m_* are 0/1 masks over the output grid.
Everything data-dependent runs on device; the host only prepares shape-derived
constants and input layouts (weight transpose, permuted/transposed copies of x).

Head tiling: hw is tiled as hw = p*J + j (partition-major) so that each
anchor's output region in HBM is contiguous per partition (J x 340B runs) —
one DMA per (group, anchor) instead of 25k 340B packets.
"""
import sys
import numpy as np
import ml_dtypes

try:
    import concourse  # noqa: F401
except ImportError:
    sys.path.insert(0, "/opt/trn_rl_repo")

import concourse.bass as bass  # noqa: E402,F401
import concourse.tile as tile  # noqa: E402
from concourse import bacc, mybir  # noqa: E402
from concourse.bass_utils import run_bass_kernel_spmd  # noqa: E402

F32 = mybir.dt.float32
F32R = mybir.dt.float32r
BF16 = mybir.dt.bfloat16
BF = ml_dtypes.bfloat16

B, N = 8, 64
NC_, NA = 80, 3
NO = NC_ + 5          # 85
NOP = 256             # padded head output channels
CH = (128, 256, 512)
HW = (80, 40, 20)
S = 100
PJ = ((128, 50), (100, 16), (100, 4))   # (P, J): hw = p*J + j, exact covers H*H
GH = 4                                   # head tiles per psum group / out-DMA


def _resize_matrix(h, s, method):
    scale = s / h
    x = (np.arange(s, dtype=np.float64) + 0.5) / scale - 0.5
    d = x[:, None] - np.arange(h, dtype=np.float64)[None, :]
    ad = np.abs(d)
    if method == "linear":
        w = np.clip(1.0 - ad, 0.0, None)
    else:  # Keys cubic, a = -0.5
        a = -0.5
        w = np.where(
            ad <= 1.0,
            ((a + 2.0) * ad - (a + 3.0)) * ad * ad + 1.0,
            np.where(ad < 2.0, ((a * ad - 5.0 * a) * ad + 8.0 * a) * ad - 4.0 * a, 0.0),
        )
    w = w / w.sum(axis=1, keepdims=True)
    return np.ascontiguousarray(w, dtype=np.float32)


def _ceil_div(a, b):
    return -(-a // b)


_PROGRAM = None


def _build_program(bias_nonzero=False):
    nc = bacc.Bacc("TRN2", target_bir_lowering=False, debug=False, num_devices=B)

    # x{l}: heads operand, hw-columns permuted to (j, p) order; xt{l}: bf16 X^T
    d_x = [nc.dram_tensor(f"x{l}", [128, (CH[l] // 128) * HW[l] * HW[l]], F32, kind="ExternalInput").ap() for l in range(3)]
    d_xt = [nc.dram_tensor(f"xt{l}", [128, _ceil_div(HW[l] * HW[l], 128) * CH[l]], BF16, kind="ExternalInput").ap() for l in range(3)]
    d_w = nc.dram_tensor("wblob", [128, 7 * NOP], F32, kind="ExternalInput").ap()
    d_bias = nc.dram_tensor("biasblob", [128, 3 * NOP], F32, kind="ExternalInput").ap()
    d_a = nc.dram_tensor("ablob", [S, sum(HW)], F32, kind="ExternalInput").ap()
    d_misc = nc.dram_tensor("misc", [N, S + 7], F32, kind="ExternalInput").ap()
    d_iden = nc.dram_tensor("iden", [128, 128], F32, kind="ExternalInput").ap()

    d_p = [nc.dram_tensor(f"p{l}", [NA, HW[l], HW[l], NO], F32, kind="ExternalOutput").ap() for l in range(3)]
    d_obj = nc.dram_tensor("obj", [N, sum(CH)], F32, kind="ExternalOutput").ap()

    AL = mybir.AluOpType
    SHW = 2 * sum(HW)  # 280

    with tile.TileContext(nc) as tc:
        with (
            tc.tile_pool(name="const", bufs=1) as cpool,
            tc.tile_pool(name="xin", bufs=1) as xpool,
            tc.tile_pool(name="lab", bufs=1) as lpool,
            tc.tile_pool(name="wmap", bufs=2) as wmpool,
            tc.tile_pool(name="hsb", bufs=4) as hsbpool,
            tc.tile_pool(name="xtsb", bufs=3) as xtsbpool,
            tc.tile_pool(name="hps", bufs=2, space="PSUM") as hpspool,      # 2 banks x 2
            tc.tile_pool(name="xtps", bufs=2, space="PSUM") as xtpspool,    # 1 bank x 2
            tc.tile_pool(name="ops", bufs=1, space="PSUM") as opspool,      # 1 bank
            tc.tile_pool(name="mps", bufs=1, space="PSUM") as mpspool,      # 1 bank
        ):
            # ---- loads: packed const blobs, then x chunks; xt via HWDGE
            wblob = cpool.tile([128, 7 * NOP], F32R, tag="wblob")
            nc.gpsimd.dma_start(wblob[:], d_w[:])
            w_t = []
            woff = 0
            for l in range(3):
                blocks = []
                for rb in range(CH[l] // 128):
                    blocks.append(wblob[:, woff * NOP:(woff + 1) * NOP])
                    woff += 1
                w_t.append(blocks)
            misc_t = cpool.tile([N, S + 7], F32, tag="misc")
            nc.gpsimd.dma_start(misc_t[:], d_misc[:])
            grid_t = misc_t[:, 0:S]
            lab_t = misc_t[:, S:S + 6]
            myid_t = misc_t[:, S + 6:S + 7]
            ablob = cpool.tile([S, sum(HW)], F32R, tag="ablob")
            nc.gpsimd.dma_start(ablob[:], d_a[:])
            a_t = [ablob[:, 0:80], ablob[:, 80:120], ablob[:, 120:140]]
            iden_t = cpool.tile([128, 128], F32R, tag="iden")
            nc.gpsimd.dma_start(iden_t[:], d_iden[:])
            idenb_t = cpool.tile([N, N], BF16, tag="idenb")
            nc.gpsimd.dma_start(idenb_t[:], d_iden[0:N, 0:N])
            x_t = []
            for l in range(3):
                hw2 = HW[l] * HW[l]
                RB = CH[l] // 128
                t = xpool.tile([128, RB * hw2], F32R, tag=f"x{l}")
                nch = 4 if l == 0 else 1
                step = RB * hw2 // nch
                assert step * nch == RB * hw2
                for c in range(nch):
                    nc.gpsimd.dma_start(
                        t[:, c * step:(c + 1) * step],
                        d_x[l][:, c * step:(c + 1) * step],
                    )
                x_t.append([t[:, rb * hw2:(rb + 1) * hw2] for rb in range(RB)])
            xt_t = []
            for l in range(3):
                Tl = _ceil_div(HW[l] * HW[l], 128)
                t = cpool.tile([128, Tl * CH[l]], BF16, tag=f"xt{l}")
                nc.sync.dma_start(t[:], d_xt[l][:])
                xt_t.append(t)
            biasblob = cpool.tile([128, 3 * NOP], F32, tag="biasblob")
            nc.gpsimd.dma_start(biasblob[:], d_bias[:])
            bias_t = [biasblob[:, l * NOP:(l + 1) * NOP] for l in range(3)]

            # ---- label processing -> masks (DVE)
            xc, yc, bw, bh = (lab_t[:, i:i + 1] for i in (2, 3, 4, 5))
            halfw = lpool.tile([N, 1], F32, tag="halfw")
            halfh = lpool.tile([N, 1], F32, tag="halfh")
            nc.vector.tensor_scalar(halfw[:], bw, 0.5, None, AL.mult)
            nc.vector.tensor_scalar(halfh[:], bh, 0.5, None, AL.mult)
            v1x = lpool.tile([N, 1], F32, tag="v1x")
            v2x = lpool.tile([N, 1], F32, tag="v2x")
            v1y = lpool.tile([N, 1], F32, tag="v1y")
            v2y = lpool.tile([N, 1], F32, tag="v2y")
            nc.vector.tensor_scalar(v1x[:], xc, halfw[:, 0:1], float(S), AL.subtract, AL.mult)
            nc.vector.tensor_scalar(v2x[:], xc, halfw[:, 0:1], float(S), AL.add, AL.mult)
            nc.vector.tensor_scalar(v1y[:], yc, halfh[:, 0:1], float(S), AL.subtract, AL.mult)
            nc.vector.tensor_scalar(v2y[:], yc, halfh[:, 0:1], float(S), AL.add, AL.mult)
            mx_a = lpool.tile([N, S], F32, tag="mx_a")
            mx_b = lpool.tile([N, S], F32, tag="mx_b")
            my_a = lpool.tile([N, S], F32, tag="my_a")
            my_b = lpool.tile([N, S], F32, tag="my_b")
            nc.vector.tensor_scalar(mx_a[:], grid_t[:], v1x[:, 0:1], None, AL.is_gt)
            nc.vector.tensor_scalar(mx_b[:], grid_t[:], v2x[:, 0:1], None, AL.is_le)
            nc.vector.tensor_scalar(my_a[:], grid_t[:], v1y[:, 0:1], None, AL.is_gt)
            nc.vector.tensor_scalar(my_b[:], grid_t[:], v2y[:, 0:1], None, AL.is_le)
            m_x = lpool.tile([N, S], F32, tag="m_x")
            m_y = lpool.tile([N, S], F32, tag="m_y")
            nc.vector.tensor_mul(m_x[:], mx_a[:], mx_b[:])
            nc.vector.tensor_mul(m_y[:], my_a[:], my_b[:])
            ax = lpool.tile([N, 1], F32, tag="ax")
            ay = lpool.tile([N, 1], F32, tag="ay")
            nc.vector.tensor_reduce(ax[:], m_x[:], mybir.AxisListType.X, AL.add)
            nc.vector.tensor_reduce(ay[:], m_y[:], mybir.AxisListType.X, AL.add)
            area = lpool.tile([N, 1], F32, tag="area")
            nc.vector.tensor_mul(area[:], ax[:], ay[:])
            recip = lpool.tile([N, 1], F32, tag="recip")
            nc.vector.reciprocal(recip[:], area[:])
            ind = lpool.tile([N, 1], F32, tag="ind")
            nc.vector.tensor_scalar(ind[:], lab_t[:, 0:1], myid_t[:, 0:1], None, AL.is_equal)
            mxs = lpool.tile([N, S], F32R, tag="mxs")   # x-mask gated by indicator
            mys = lpool.tile([N, S], F32R, tag="mys")
            nc.vector.tensor_scalar(mxs[:], m_x[:], ind[:, 0:1], None, AL.mult)
            nc.vector.tensor_scalar(mys[:], m_y[:], 1.0, None, AL.mult)

            # ---- early PE: mask transposes + R/C matmuls (one PSUM bank)
            early_ps = mpspool.tile([S, 2 * N + SHW], F32R, tag="early_ps")
            nc.tensor.transpose(early_ps[:, 0:N], mxs[:], iden_t[0:N, 0:N])
            nc.tensor.transpose(early_ps[:, N:2 * N], mys[:], iden_t[0:N, 0:N])
            mT = lpool.tile([S, 2 * N], F32R, tag="mT")  # [:, :N]=mxT, [:, N:]=myT
            nc.scalar.copy(mT[:], early_ps[:, 0:2 * N])
            offs = []
            off = 0
            for l in range(3):
                nc.tensor.matmul(early_ps[0:N, 2 * N + off:2 * N + off + HW[l]].bitcast(F32),
                                 mT[:, N:2 * N], a_t[l][:], start=True, stop=True)
                nc.tensor.matmul(early_ps[0:N, 2 * N + off + HW[l]:2 * N + off + 2 * HW[l]].bitcast(F32),
                                 mT[:, 0:N], a_t[l][:], start=True, stop=True)
                offs.append(off)
                off += 2 * HW[l]
            rc = lpool.tile([N, SHW], BF16, tag="rc")   # per level: [R | C_gated]
            nc.vector.tensor_copy(rc[:], early_ps[0:N, 2 * N:2 * N + SHW].bitcast(F32))

            # ---- heads first (all levels): PE starts as soon as x chunks land
            use_act_evac = not bias_nonzero
            evac_flip = [0]
            for l in range(3):
                C = CH[l]
                RB = C // 128
                P, J = PJ[l]
                GW = 8 if J % 8 == 0 or J > 8 else J
                for w0 in range(0, J, GW):
                    w1 = min(w0 + GW, J)
                    h_sb = hsbpool.tile([128, NA * GW * NO], F32, tag="h_sb")
                    for g0 in range(w0, w1, GH):
                        g1 = min(g0 + GH, w1)
                        ng = g1 - g0
                        h_ps = hpspool.tile([128, GH * NOP], F32, tag="h_ps")
                        for jt in range(g0, g1):
                            jj = jt - g0
                            for rb in range(RB):
                                nc.tensor.matmul(
                                    h_ps[0:P, jj * NOP:(jj + 1) * NOP],
                                    x_t[l][rb][:, jt * P:(jt + 1) * P],
                                    w_t[l][rb][:],
                                    start=(rb == 0),
                                    stop=(rb == RB - 1),
                                )
                        # evac to anchor-major layout h_sb[p, (a, j, o)]
                        out_ap = h_sb[0:P, :].rearrange(
                            "p (a j o) -> p a j o", a=NA, j=GW
                        )[:, :, g0 - w0:g1 - w0, :]
                        in_ap = h_ps[0:P, 0:ng * NOP].rearrange(
                            "p (j o) -> p j o", o=NOP
                        )[:, :, 0:NA * NO].rearrange("p j (a o) -> p a j o", a=NA)
                        if use_act_evac and evac_flip[0] % 2 == 0:
                            nc.scalar.copy(out_ap, in_ap)
                        else:
                            nc.vector.tensor_tensor(
                                out_ap,
                                in_ap,
                                bias_t[l][0:P, 0:NA * NO].rearrange(
                                    "p (a o) -> p a o", a=NA
                                ).unsqueeze(2).broadcast_to([P, NA, ng, NO]),
                                AL.add,
                            )
                        evac_flip[0] += 1
                    pflat = d_p[l].rearrange("a h w o -> a (h w o)")
                    for a in range(NA):
                        nc.sync.dma_start(
                            pflat[a].rearrange("(p j o) -> p j o", p=P, o=NO)[:, w0:w1, :],
                            h_sb[0:P, a * GW * NO:a * GW * NO + (w1 - w0) * NO],
                        )

            # ---- pooling per level
            obj_sb = wmpool.tile([N, sum(CH)], F32, tag="objsb")
            for l in range(3):
                H = HW[l]
                HW2 = H * H
                C = CH[l]
                T = _ceil_div(HW2, 128)
                r_vec = rc[:, offs[l]:offs[l] + H]
                c_vec = rc[:, offs[l] + H:offs[l] + 2 * H]

                wmap = wmpool.tile([N, HW2], BF16, tag="wmap")
                nc.vector.tensor_tensor(
                    wmap[:].rearrange("p (a b) -> p a b", a=H),
                    r_vec.unsqueeze(2).broadcast_to([N, H, H]),
                    c_vec.unsqueeze(1).broadcast_to([N, H, H]),
                    AL.mult,
                )
                wmapT = wmpool.tile([128, T * N], BF16, tag="wmapT")
                for g0 in range(0, T, 8):
                    g1 = min(g0 + 8, T)
                    wt_ps = xtpspool.tile([128, 8 * N], BF16, tag="xt_ps")
                    for t in range(g0, g1):
                        p = min(128, HW2 - t * 128)
                        nc.tensor.transpose(
                            wt_ps[0:p, (t - g0) * N:(t - g0 + 1) * N],
                            wmap[:, t * 128:t * 128 + p],
                            idenb_t[:],
                        )
                    nc.scalar.copy(wmapT[:, g0 * N:g1 * N], wt_ps[:, 0:(g1 - g0) * N])

                obj_ps = opspool.tile([N, C], F32, tag="obj")
                for t in range(T):
                    nc.tensor.matmul(
                        obj_ps[:, :],
                        wmapT[:, t * N:(t + 1) * N],
                        xt_t[l][:, t * C:(t + 1) * C],
                        start=(t == 0),
                        stop=(t == T - 1),
                    )
                coff = sum(CH[:l])
                nc.vector.tensor_scalar(obj_sb[:, coff:coff + C], obj_ps[:], recip[:, 0:1], None, AL.mult)
                if l == 2:
                    nc.sync.dma_start(d_obj[:], obj_sb[:])

    nc.compile()
    return nc


def _make_consts():
    a_mats = [
        _resize_matrix(HW[0], S, "linear"),
        _resize_matrix(HW[1], S, "cubic"),
        _resize_matrix(HW[2], S, "cubic"),
    ]
    grid = np.tile((np.arange(S, dtype=np.float32) + np.float32(0.5))[None, :], (N, 1))
    iden = np.eye(128, dtype=np.float32)
    return a_mats, grid, iden


def _make_in_maps(x0, x1, x2, labels, w0, b0, w1, b1, w2, b2):
    a_mats, grid, iden = _make_consts()
    xs = [np.asarray(x0), np.asarray(x1), np.asarray(x2)]
    ws = [np.asarray(w0), np.asarray(w1), np.asarray(w2)]
    bs = [np.asarray(b0), np.asarray(b1), np.asarray(b2)]
    labels = np.asarray(labels)

    wblob = np.zeros((128, 7 * NOP), dtype=np.float32)
    woff = 0
    for l in range(3):
        wt = ws[l].T  # [C, 255]
        for rb in range(CH[l] // 128):
            wblob[:, woff * NOP:woff * NOP + NO * NA] = wt[rb * 128:(rb + 1) * 128]
            woff += 1
    biasblob = np.zeros((128, 3 * NOP), dtype=np.float32)
    for l in range(3):
        biasblob[:, l * NOP:l * NOP + NO * NA] = np.tile(bs[l][None, :], (128, 1))
    ablob = np.concatenate(a_mats, axis=1).astype(np.float32)
    base = {
        "wblob": wblob,
        "biasblob": biasblob,
        "ablob": np.ascontiguousarray(ablob),
        "iden": iden,
    }

    in_maps = []
    for b in range(B):
        m = dict(base)
        misc = np.zeros((N, S + 7), dtype=np.float32)
        misc[:, 0:S] = grid
        misc[:, S:S + 6] = labels.astype(np.float32)
        misc[:, S + 6] = float(b)
        m["misc"] = misc
        for l in range(3):
            P, J = PJ[l]
            hw2 = HW[l] * HW[l]
            RB = CH[l] // 128
            xf = xs[l][b].reshape(CH[l], hw2).astype(np.float32)
            # permute columns to (j, p) blocks: column j*P+p holds hw = p*J+j
            xperm = xf.reshape(CH[l], P, J).transpose(0, 2, 1).reshape(CH[l], hw2)
            m[f"x{l}"] = np.ascontiguousarray(
                xperm.reshape(RB, 128, hw2).transpose(1, 0, 2).reshape(128, RB * hw2)
            )
            Tl = _ceil_div(hw2, 128)
            xtp = np.zeros((Tl * 128, CH[l]), dtype=BF)
            xtp[:hw2] = xf.T.astype(BF)
            m[f"xt{l}"] = np.ascontiguousarray(
                xtp.reshape(Tl, 128, CH[l]).transpose(1, 0, 2).reshape(128, Tl * CH[l])
            )
        in_maps.append(m)
    return in_maps


_PROGRAMS = {}


def kernel(x0, x1, x2, labels, w0, b0, w1, b1, w2, b2):
    bias_nonzero = any(np.any(np.asarray(b)) for b in (b0, b1, b2))
    if bias_nonzero not in _PROGRAMS:
        _PROGRAMS[bias_nonzero] = _build_program(bias_nonzero)
    nc = _PROGRAMS[bias_nonzero]

    labels = np.asarray(labels)
    in_maps = _make_in_maps(x0, x1, x2, labels, w0, b0, w1, b1, w2, b2)
    res = run_bass_kernel_spmd(nc, in_maps, list(range(B)))

    p_out = [
        np.stack([res.results[b][f"p{l}"] for b in range(B)], axis=0) for l in range(3)
    ]
    obj = np.zeros((N, sum(CH)), dtype=np.float32)
    for b in range(B):
        obj += res.results[b]["obj"]
    gt = np.ascontiguousarray(labels[:, 1], dtype=np.float32)
    return (p_out[0], p_out[1], p_out[2], obj, gt)


# revision 12
# speedup vs baseline: 1.2854x; 1.2854x over previous
"""Trainium2 Bass kernel for nn_Detect (YOLO-style heads + ROI-pooled obj features).

Sharding: data-parallel over batch — core b owns image b. Boxes are selected
per-core via an on-device indicator (labels[:,0] == core_id); the per-core obj
partial outputs are disjoint row-wise and summed on the host (unshard).

Math: the reference's upsample(bilinear/bicubic to 100x100) -> SAT -> box-sum
pipeline is linear and separable, so per box n and level l:
    obj[n, c] = (1/area_n) * r_n^T  X_l[b_n, c]  c_n
with r_n = A_l^T m_y_n, c_n = A_l^T m_x_n, where A_l is the (constant) 1-D
interpolation matrix [100, H_l] and m_* are 0/1 masks over the output grid.
Everything data-dependent runs on device; the host only prepares shape-derived
constants and input layouts (weight transpose, permuted/transposed copies of x).

Head tiling: hw is tiled as hw = p*J + j (partition-major) so that each
anchor's output region in HBM is contiguous per partition (J x 340B runs) —
one DMA per (group, anchor) instead of 25k 340B packets.
"""
import sys
import numpy as np
import ml_dtypes

try:
    import concourse  # noqa: F401
except ImportError:
    sys.path.insert(0, "/opt/trn_rl_repo")

import concourse.bass as bass  # noqa: E402,F401
import concourse.tile as tile  # noqa: E402
from concourse import bacc, mybir  # noqa: E402
from concourse.bass_utils import run_bass_kernel_spmd  # noqa: E402

F32 = mybir.dt.float32
F32R = mybir.dt.float32r
BF16 = mybir.dt.bfloat16
BF = ml_dtypes.bfloat16

B, N = 8, 64
NC_, NA = 80, 3
NO = NC_ + 5          # 85
NOP = 256             # padded head output channels
CH = (128, 256, 512)
HW = (80, 40, 20)
S = 100
PJ = ((128, 50), (100, 16), (100, 4))   # (P, J): hw = p*J + j, exact covers H*H
GH = 4                                   # head tiles per psum group / out-DMA


def _resize_matrix(h, s, method):
    scale = s / h
    x = (np.arange(s, dtype=np.float64) + 0.5) / scale - 0.5
    d = x[:, None] - np.arange(h, dtype=np.float64)[None, :]
    ad = np.abs(d)
    if method == "linear":
        w = np.clip(1.0 - ad, 0.0, None)
    else:  # Keys cubic, a = -0.5
        a = -0.5
        w = np.where(
            ad <= 1.0,
            ((a + 2.0) * ad - (a + 3.0)) * ad * ad + 1.0,
            np.where(ad < 2.0, ((a * ad - 5.0 * a) * ad + 8.0 * a) * ad - 4.0 * a, 0.0),
        )
    w = w / w.sum(axis=1, keepdims=True)
    return np.ascontiguousarray(w, dtype=np.float32)


def _ceil_div(a, b):
    return -(-a // b)


_PROGRAM = None


def _build_program(bias_nonzero=False):
    nc = bacc.Bacc("TRN2", target_bir_lowering=False, debug=False, num_devices=B)

    # x{l}: heads operand, hw-columns permuted to (j, p) order; xt{l}: bf16 X^T
    d_x = [nc.dram_tensor(f"x{l}", [128, (CH[l] // 128) * HW[l] * HW[l]], F32, kind="ExternalInput").ap() for l in range(3)]
    d_xt = [nc.dram_tensor(f"xt{l}", [128, _ceil_div(HW[l] * HW[l], 128) * CH[l]], BF16, kind="ExternalInput").ap() for l in range(3)]
    d_w = nc.dram_tensor("wblob", [128, 7 * NOP], F32, kind="ExternalInput").ap()
    d_bias = nc.dram_tensor("biasblob", [128, 3 * NOP], F32, kind="ExternalInput").ap()
    d_a = nc.dram_tensor("ablob", [S, sum(HW)], F32, kind="ExternalInput").ap()
    d_misc = nc.dram_tensor("misc", [N, S + 7], F32, kind="ExternalInput").ap()
    d_iden = nc.dram_tensor("iden", [128, 128], F32, kind="ExternalInput").ap()

    d_p = [nc.dram_tensor(f"p{l}", [NA, HW[l], HW[l], NO], F32, kind="ExternalOutput").ap() for l in range(3)]
    d_obj = nc.dram_tensor("obj", [N, sum(CH)], F32, kind="ExternalOutput").ap()

    AL = mybir.AluOpType
    SHW = 2 * sum(HW)  # 280

    with tile.TileContext(nc) as tc:
        with (
            tc.tile_pool(name="const", bufs=1) as cpool,
            tc.tile_pool(name="xin", bufs=1) as xpool,
            tc.tile_pool(name="lab", bufs=1) as lpool,
            tc.tile_pool(name="wmap", bufs=2) as wmpool,
            tc.tile_pool(name="hsb", bufs=4) as hsbpool,
            tc.tile_pool(name="xtsb", bufs=3) as xtsbpool,
            tc.tile_pool(name="hps", bufs=2, space="PSUM") as hpspool,      # 2 banks x 2
            tc.tile_pool(name="xtps", bufs=2, space="PSUM") as xtpspool,    # 1 bank x 2
            tc.tile_pool(name="ops", bufs=1, space="PSUM") as opspool,      # 1 bank
            tc.tile_pool(name="mps", bufs=1, space="PSUM") as mpspool,      # 1 bank
        ):
            # ---- loads: packed const blobs, then x chunks; xt via HWDGE
            wblob = cpool.tile([128, 7 * NOP], F32R, tag="wblob")
            nc.gpsimd.dma_start(wblob[:], d_w[:])
            w_t = []
            woff = 0
            for l in range(3):
                blocks = []
                for rb in range(CH[l] // 128):
                    blocks.append(wblob[:, woff * NOP:(woff + 1) * NOP])
                    woff += 1
                w_t.append(blocks)
            misc_t = cpool.tile([N, S + 7], F32, tag="misc")
            nc.gpsimd.dma_start(misc_t[:], d_misc[:])
            grid_t = misc_t[:, 0:S]
            lab_t = misc_t[:, S:S + 6]
            myid_t = misc_t[:, S + 6:S + 7]
            ablob = cpool.tile([S, sum(HW)], F32R, tag="ablob")
            nc.gpsimd.dma_start(ablob[:], d_a[:])
            a_t = [ablob[:, 0:80], ablob[:, 80:120], ablob[:, 120:140]]
            iden_t = cpool.tile([128, 128], F32R, tag="iden")
            nc.gpsimd.dma_start(iden_t[:], d_iden[:])
            idenb_t = cpool.tile([N, N], BF16, tag="idenb")
            nc.gpsimd.dma_start(idenb_t[:], d_iden[0:N, 0:N])
            biasblob = cpool.tile([128, 3 * NOP], F32, tag="biasblob")
            nc.gpsimd.dma_start(biasblob[:], d_bias[:])
            bias_t = [biasblob[:, l * NOP:(l + 1) * NOP] for l in range(3)]
            # x0 split into window-aligned chunk tiles so heads start early
            x0_bounds = [0, 13 * 128, 26 * 128, 38 * 128, 50 * 128]
            x0_tiles = []
            for c in range(4):
                lo, hi = x0_bounds[c], x0_bounds[c + 1]
                t = xpool.tile([128, hi - lo], F32R, tag=f"x0_{c}")
                nc.gpsimd.dma_start(t[:], d_x[0][:, lo:hi])
                x0_tiles.append(t)

            def x0_win(j0, j1):  # columns j0*128:j1*128 of permuted x0
                c = next(i for i in range(4) if x0_bounds[i + 1] >= j1 * 128)
                return x0_tiles[c][:, j0 * 128 - x0_bounds[c]:j1 * 128 - x0_bounds[c]]

            x_t = [None]
            for l in (1, 2):
                hw2 = HW[l] * HW[l]
                RB = CH[l] // 128
                t = xpool.tile([128, RB * hw2], F32R, tag=f"x{l}")
                nc.gpsimd.dma_start(t[:], d_x[l][:])
                x_t.append([t[:, rb * hw2:(rb + 1) * hw2] for rb in range(RB)])
            xt_t = []
            for l in range(3):
                Tl = _ceil_div(HW[l] * HW[l], 128)
                t = cpool.tile([128, Tl * CH[l]], BF16, tag=f"xt{l}")
                nc.sync.dma_start(t[:], d_xt[l][:])
                xt_t.append(t)

            # ---- label processing -> masks (DVE)
            xc, yc, bw, bh = (lab_t[:, i:i + 1] for i in (2, 3, 4, 5))
            halfw = lpool.tile([N, 1], F32, tag="halfw")
            halfh = lpool.tile([N, 1], F32, tag="halfh")
            nc.vector.tensor_scalar(halfw[:], bw, 0.5, None, AL.mult)
            nc.vector.tensor_scalar(halfh[:], bh, 0.5, None, AL.mult)
            v1x = lpool.tile([N, 1], F32, tag="v1x")
            v2x = lpool.tile([N, 1], F32, tag="v2x")
            v1y = lpool.tile([N, 1], F32, tag="v1y")
            v2y = lpool.tile([N, 1], F32, tag="v2y")
            nc.vector.tensor_scalar(v1x[:], xc, halfw[:, 0:1], float(S), AL.subtract, AL.mult)
            nc.vector.tensor_scalar(v2x[:], xc, halfw[:, 0:1], float(S), AL.add, AL.mult)
            nc.vector.tensor_scalar(v1y[:], yc, halfh[:, 0:1], float(S), AL.subtract, AL.mult)
            nc.vector.tensor_scalar(v2y[:], yc, halfh[:, 0:1], float(S), AL.add, AL.mult)
            mx_a = lpool.tile([N, S], F32, tag="mx_a")
            mx_b = lpool.tile([N, S], F32, tag="mx_b")
            my_a = lpool.tile([N, S], F32, tag="my_a")
            my_b = lpool.tile([N, S], F32, tag="my_b")
            nc.vector.tensor_scalar(mx_a[:], grid_t[:], v1x[:, 0:1], None, AL.is_gt)
            nc.vector.tensor_scalar(mx_b[:], grid_t[:], v2x[:, 0:1], None, AL.is_le)
            nc.vector.tensor_scalar(my_a[:], grid_t[:], v1y[:, 0:1], None, AL.is_gt)
            nc.vector.tensor_scalar(my_b[:], grid_t[:], v2y[:, 0:1], None, AL.is_le)
            m_x = lpool.tile([N, S], F32, tag="m_x")
            m_y = lpool.tile([N, S], F32, tag="m_y")
            nc.vector.tensor_mul(m_x[:], mx_a[:], mx_b[:])
            nc.vector.tensor_mul(m_y[:], my_a[:], my_b[:])
            ax = lpool.tile([N, 1], F32, tag="ax")
            ay = lpool.tile([N, 1], F32, tag="ay")
            nc.vector.tensor_reduce(ax[:], m_x[:], mybir.AxisListType.X, AL.add)
            nc.vector.tensor_reduce(ay[:], m_y[:], mybir.AxisListType.X, AL.add)
            area = lpool.tile([N, 1], F32, tag="area")
            nc.vector.tensor_mul(area[:], ax[:], ay[:])
            recip = lpool.tile([N, 1], F32, tag="recip")
            nc.vector.reciprocal(recip[:], area[:])
            ind = lpool.tile([N, 1], F32, tag="ind")
            nc.vector.tensor_scalar(ind[:], lab_t[:, 0:1], myid_t[:, 0:1], None, AL.is_equal)
            mxs = lpool.tile([N, S], F32R, tag="mxs")   # x-mask gated by indicator
            mys = lpool.tile([N, S], F32R, tag="mys")
            nc.vector.tensor_scalar(mxs[:], m_x[:], ind[:, 0:1], None, AL.mult)
            nc.vector.tensor_scalar(mys[:], m_y[:], 1.0, None, AL.mult)

            # ---- early PE: mask transposes + R/C matmuls (one PSUM bank)
            early_ps = mpspool.tile([S, 2 * N + SHW], F32R, tag="early_ps")
            nc.tensor.transpose(early_ps[:, 0:N], mxs[:], iden_t[0:N, 0:N])
            nc.tensor.transpose(early_ps[:, N:2 * N], mys[:], iden_t[0:N, 0:N])
            mT = lpool.tile([S, 2 * N], F32R, tag="mT")  # [:, :N]=mxT, [:, N:]=myT
            nc.scalar.copy(mT[:], early_ps[:, 0:2 * N])
            offs = []
            off = 0
            for l in range(3):
                nc.tensor.matmul(early_ps[0:N, 2 * N + off:2 * N + off + HW[l]].bitcast(F32),
                                 mT[:, N:2 * N], a_t[l][:], start=True, stop=True)
                nc.tensor.matmul(early_ps[0:N, 2 * N + off + HW[l]:2 * N + off + 2 * HW[l]].bitcast(F32),
                                 mT[:, 0:N], a_t[l][:], start=True, stop=True)
                offs.append(off)
                off += 2 * HW[l]
            rc = lpool.tile([N, SHW], BF16, tag="rc")   # per level: [R | C_gated]
            nc.vector.tensor_copy(rc[:], early_ps[0:N, 2 * N:2 * N + SHW].bitcast(F32))

            # ---- heads first (all levels): PE starts as soon as x chunks land
            use_act_evac = not bias_nonzero
            evac_flip = [0]
            for l in range(3):
                C = CH[l]
                RB = C // 128
                P, J = PJ[l]
                GW = 8 if J % 8 == 0 or J > 8 else J
                for w0 in range(0, J, GW):
                    w1 = min(w0 + GW, J)
                    h_sb = hsbpool.tile([128, NA * GW * NO], F32, tag="h_sb")
                    for g0 in range(w0, w1, GH):
                        g1 = min(g0 + GH, w1)
                        ng = g1 - g0
                        h_ps = hpspool.tile([128, GH * NOP], F32, tag="h_ps")
                        for jt in range(g0, g1):
                            jj = jt - g0
                            for rb in range(RB):
                                lhs = (
                                    x0_win(jt, jt + 1)
                                    if l == 0
                                    else x_t[l][rb][:, jt * P:(jt + 1) * P]
                                )
                                nc.tensor.matmul(
                                    h_ps[0:P, jj * NOP:(jj + 1) * NOP],
                                    lhs,
                                    w_t[l][rb][:],
                                    start=(rb == 0),
                                    stop=(rb == RB - 1),
                                )
                        # evac to anchor-major layout h_sb[p, (a, j, o)]
                        out_ap = h_sb[0:P, :].rearrange(
                            "p (a j o) -> p a j o", a=NA, j=GW
                        )[:, :, g0 - w0:g1 - w0, :]
                        in_ap = h_ps[0:P, 0:ng * NOP].rearrange(
                            "p (j o) -> p j o", o=NOP
                        )[:, :, 0:NA * NO].rearrange("p j (a o) -> p a j o", a=NA)
                        if use_act_evac and evac_flip[0] % 2 == 0:
                            nc.scalar.copy(out_ap, in_ap)
                        else:
                            nc.vector.tensor_tensor(
                                out_ap,
                                in_ap,
                                bias_t[l][0:P, 0:NA * NO].rearrange(
                                    "p (a o) -> p a o", a=NA
                                ).unsqueeze(2).broadcast_to([P, NA, ng, NO]),
                                AL.add,
                            )
                        evac_flip[0] += 1
                    pflat = d_p[l].rearrange("a h w o -> a (h w o)")
                    for a in range(NA):
                        nc.sync.dma_start(
                            pflat[a].rearrange("(p j o) -> p j o", p=P, o=NO)[:, w0:w1, :],
                            h_sb[0:P, a * GW * NO:a * GW * NO + (w1 - w0) * NO],
                        )

            # ---- pooling per level
            obj_sb = wmpool.tile([N, sum(CH)], F32, tag="objsb")
            for l in range(3):
                H = HW[l]
                HW2 = H * H
                C = CH[l]
                T = _ceil_div(HW2, 128)
                r_vec = rc[:, offs[l]:offs[l] + H]
                c_vec = rc[:, offs[l] + H:offs[l] + 2 * H]

                wmap = wmpool.tile([N, HW2], BF16, tag="wmap")
                nc.vector.tensor_tensor(
                    wmap[:].rearrange("p (a b) -> p a b", a=H),
                    r_vec.unsqueeze(2).broadcast_to([N, H, H]),
                    c_vec.unsqueeze(1).broadcast_to([N, H, H]),
                    AL.mult,
                )
                wmapT = wmpool.tile([128, T * N], BF16, tag="wmapT")
                for g0 in range(0, T, 8):
                    g1 = min(g0 + 8, T)
                    wt_ps = xtpspool.tile([128, 8 * N], BF16, tag="xt_ps")
                    for t in range(g0, g1):
                        p = min(128, HW2 - t * 128)
                        nc.tensor.transpose(
                            wt_ps[0:p, (t - g0) * N:(t - g0 + 1) * N],
                            wmap[:, t * 128:t * 128 + p],
                            idenb_t[:],
                        )
                    nc.scalar.copy(wmapT[:, g0 * N:g1 * N], wt_ps[:, 0:(g1 - g0) * N])

                obj_ps = opspool.tile([N, C], F32, tag="obj")
                for t in range(T):
                    nc.tensor.matmul(
                        obj_ps[:, :],
                        wmapT[:, t * N:(t + 1) * N],
                        xt_t[l][:, t * C:(t + 1) * C],
                        start=(t == 0),
                        stop=(t == T - 1),
                    )
                coff = sum(CH[:l])
                nc.vector.tensor_scalar(obj_sb[:, coff:coff + C], obj_ps[:], recip[:, 0:1], None, AL.mult)
                if l == 2:
                    nc.sync.dma_start(d_obj[:], obj_sb[:])

    nc.compile()
    return nc


def _make_consts():
    a_mats = [
        _resize_matrix(HW[0], S, "linear"),
        _resize_matrix(HW[1], S, "cubic"),
        _resize_matrix(HW[2], S, "cubic"),
    ]
    grid = np.tile((np.arange(S, dtype=np.float32) + np.float32(0.5))[None, :], (N, 1))
    iden = np.eye(128, dtype=np.float32)
    return a_mats, grid, iden


def _make_in_maps(x0, x1, x2, labels, w0, b0, w1, b1, w2, b2):
    a_mats, grid, iden = _make_consts()
    xs = [np.asarray(x0), np.asarray(x1), np.asarray(x2)]
    ws = [np.asarray(w0), np.asarray(w1), np.asarray(w2)]
    bs = [np.asarray(b0), np.asarray(b1), np.asarray(b2)]
    labels = np.asarray(labels)

    wblob = np.zeros((128, 7 * NOP), dtype=np.float32)
    woff = 0
    for l in range(3):
        wt = ws[l].T  # [C, 255]
        for rb in range(CH[l] // 128):
            wblob[:, woff * NOP:woff * NOP + NO * NA] = wt[rb * 128:(rb + 1) * 128]
            woff += 1
    biasblob = np.zeros((128, 3 * NOP), dtype=np.float32)
    for l in range(3):
        biasblob[:, l * NOP:l * NOP + NO * NA] = np.tile(bs[l][None, :], (128, 1))
    ablob = np.concatenate(a_mats, axis=1).astype(np.float32)
    base = {
        "wblob": wblob,
        "biasblob": biasblob,
        "ablob": np.ascontiguousarray(ablob),
        "iden": iden,
    }

    in_maps = []
    for b in range(B):
        m = dict(base)
        misc = np.zeros((N, S + 7), dtype=np.float32)
        misc[:, 0:S] = grid
        misc[:, S:S + 6] = labels.astype(np.float32)
        misc[:, S + 6] = float(b)
        m["misc"] = misc
        for l in range(3):
            P, J = PJ[l]
            hw2 = HW[l] * HW[l]
            RB = CH[l] // 128
            xf = xs[l][b].reshape(CH[l], hw2).astype(np.float32)
            # permute columns to (j, p) blocks: column j*P+p holds hw = p*J+j
            xperm = xf.reshape(CH[l], P, J).transpose(0, 2, 1).reshape(CH[l], hw2)
            m[f"x{l}"] = np.ascontiguousarray(
                xperm.reshape(RB, 128, hw2).transpose(1, 0, 2).reshape(128, RB * hw2)
            )
            Tl = _ceil_div(hw2, 128)
            xtp = np.zeros((Tl * 128, CH[l]), dtype=BF)
            xtp[:hw2] = xf.T.astype(BF)
            m[f"xt{l}"] = np.ascontiguousarray(
                xtp.reshape(Tl, 128, CH[l]).transpose(1, 0, 2).reshape(128, Tl * CH[l])
            )
        in_maps.append(m)
    return in_maps


_PROGRAMS = {}


def kernel(x0, x1, x2, labels, w0, b0, w1, b1, w2, b2):
    bias_nonzero = any(np.any(np.asarray(b)) for b in (b0, b1, b2))
    if bias_nonzero not in _PROGRAMS:
        _PROGRAMS[bias_nonzero] = _build_program(bias_nonzero)
    nc = _PROGRAMS[bias_nonzero]

    labels = np.asarray(labels)
    in_maps = _make_in_maps(x0, x1, x2, labels, w0, b0, w1, b1, w2, b2)
    res = run_bass_kernel_spmd(nc, in_maps, list(range(B)))

    p_out = [
        np.stack([res.results[b][f"p{l}"] for b in range(B)], axis=0) for l in range(3)
    ]
    obj = np.zeros((N, sum(CH)), dtype=np.float32)
    for b in range(B):
        obj += res.results[b]["obj"]
    gt = np.ascontiguousarray(labels[:, 1], dtype=np.float32)
    return (p_out[0], p_out[1], p_out[2], obj, gt)
